# revision 49
# baseline (speedup 1.0000x reference)
"""AttnBlock (GroupNorm + single-head self-attention + residual) on 8 NeuronCores.

Sharding: data-parallel over B (4 batches) x sequence-parallel over query
rows (2 halves of H*W=4096) = 8 shards, one per core.  Each core loads its
batch's full x[b] as [C=128, HW=4096] fp16 (channels on partitions), with
the spatial columns rotated so the core's query half is cols [0:2048).

Key algebraic restructurings vs a direct lowering:
- GroupNorm folds to a per-channel affine h = A*x + B.  A is applied once
  to x (xh = A*x, one DVE op); the K-side B is dropped (softmax row
  invariance), the Q-side B rides in hq = A*x - Bneg, the V-side B folds
  into a constant output bias.
- The K projection is eliminated entirely: scores st[j,i] = k_j . q_i =
  xh_j^T (Wk Wq^T) h_i, so a single host-side product G = Wq Wk^T (scaled
  by C^-0.5 and the Schraudolph constant SCH_A) turns the whole Q/K path
  into one projection qk = G^T hq and score matmuls directly against xh
  blocks as stationary weights.
- Scores carry the x1477 Schraudolph pre-scale, so the DVE exp tiles are a
  SINGLE tensor_scalar (add bias, clamp at 0, int16 convert = fp16 bit
  pattern); ACT exp tiles undo the scale for free via the activation's
  fused scale/bias.

Main loop (32 key blocks x 2 query halves of 1024): scores transposed
into fp32 PSUM, exp to a per-block [128,2048] e tile (both halves), PV
accumulates oT[c,i] with V stationary.  Exp runs mostly on ScalarE; 13
halves run on VectorE via the one-op Schraudolph.  The e-sum runs as
per-half 1024-wide fp16 adds on VectorE (finer ops keep the offloaded
exp tiles' tensor_scalar from queueing behind a long add, which would
hold a score PSUM slot and stall ScalarE; 2048-wide adds and GpSimd
offload both measured slower).

Epilogue per half: denominator from es via ones-matmuls -> VectorE
reciprocal -> PE transpose -> selector-matmul broadcast; the
normalization fuses into the oT PSUM evacuation, then output projection
and a fused residual-add straight from PSUM, DMA per 512 columns.

kernel() re-executes until two runs agree bit-for-bit: a fresh NEFF's
first execution has (rarely) returned corrupted data, and a corrupted
run never reproduces.
"""

import numpy as np

C = 128
HW = 4096
NQ = 2048
HALF = 1024
JB = 32
EXP_BIAS = -8.0
EPS = 1e-5
N_CORES = 8

# Schraudolph fp16 exp: bits = round(st + SCH_B0), st = s*SCH_A pre-scaled
SCH_A = 1024.0 / float(np.log(2.0))
SCH_DELTA = -44.2
SCH_B0 = 15360.0 + SCH_DELTA + EXP_BIAS * SCH_A

# wpack (f16) column offsets
_WKQ, _WV, _WO, _ONESC = 0, 128, 256, 384
_WPACK_W = 385
# fpack (f32) column offsets
_NW, _NB, _GMAP, _GMAPT = 0, 1, 2, 34
_FPACK_W = 34 + 128

# which (jb, half) exp tiles run on VectorE (one-op Schraudolph); late
# blocks stay on ScalarE so the denominator chain is never DVE-gated
_DVE_SET = ({(jb, 0) for jb in (1, 6, 9, 11, 16, 19, 21, 26)}
            | {(jb, 1) for jb in (3, 8, 13, 18, 23)})

_NC = None


def _dve_half(jb, half):
    return (jb, half) in _DVE_SET


def _pin_activation_tables():
    """Restrict the table-load chooser to natural_log_exp_and_others so the
    kernel's ACT stream (ln/exp/copy/identity) needs a single table load."""
    from concourse.hw_specs import get_activation_tables
    tabs = get_activation_tables("gen3")
    for name in list(tabs.keys()):
        if name != "natural_log_exp_and_others":
            tabs[name] = set()


def _build_program():
    import concourse.bacc as bacc
    import concourse.tile as tile
    from concourse import mybir

    f32 = mybir.dt.float32
    f16 = mybir.dt.float16
    i16 = mybir.dt.int16
    AF = mybir.ActivationFunctionType
    OP = mybir.AluOpType

    nc = bacc.Bacc("TRN2", target_bir_lowering=False, debug=False,
                   num_devices=N_CORES)
    try:
        _pin_activation_tables()
    except Exception:
        pass

    x_d = nc.declare_dram_parameter("x", [C, HW], f16, isOutput=False)
    wpack_d = nc.declare_dram_parameter("wpack", [C, _WPACK_W], f16,
                                        isOutput=False)
    fpack_d = nc.declare_dram_parameter("fpack", [C, _FPACK_W], f32,
                                        isOutput=False)
    y_d = nc.declare_dram_parameter("y", [C, NQ], f16, isOutput=True)

    with tile.TileContext(nc) as tc:
        consts = tc.alloc_tile_pool(name="consts", bufs=1)
        big = tc.alloc_tile_pool(name="big", bufs=1)
        work = tc.alloc_tile_pool(name="work", bufs=2)
        epool = tc.alloc_tile_pool(name="epool", bufs=3)
        ypool = tc.alloc_tile_pool(name="ypool", bufs=4)
        pst = tc.alloc_tile_pool(name="pst", bufs=2, space="PSUM")
        pproj = tc.alloc_tile_pool(name="pproj", bufs=2, space="PSUM")

        # ---- input DMA: x in 4 chunks across both HWDGE rings, packs on
        # the gpsimd ring (fpack first: the stats chain needs it)
        x16 = big.tile([C, HW], f16)
        for ch in range(4):
            eng = nc.sync if ch % 2 == 0 else nc.scalar
            eng.dma_start(out=x16[:, ch * 1024:(ch + 1) * 1024],
                          in_=x_d.ap()[:, ch * 1024:(ch + 1) * 1024])
        fpack_sb = consts.tile([C, _FPACK_W], f32)
        nc.gpsimd.dma_start(out=fpack_sb, in_=fpack_d.ap())
        wpack_sb = consts.tile([C, _WPACK_W], f16)
        nc.gpsimd.dma_start(out=wpack_sb, in_=wpack_d.ap())
        wkq_sb = wpack_sb[:, _WKQ:_WKQ + C]    # lhsT for qk: (Wq Wk^T)*scale
        wv_sb = wpack_sb[:, _WV:_WV + C]
        wo_sb = wpack_sb[:, _WO:_WO + C]
        onesc_sb = wpack_sb[:, _ONESC:_ONESC + 1]
        nw_sb = fpack_sb[:, _NW:_NW + 1]
        nb_sb = fpack_sb[:, _NB:_NB + 1]
        gmap_sb = fpack_sb[:, _GMAP:_GMAP + 32]
        gmapt_sb = fpack_sb[0:32, _GMAPT:_GMAPT + C]

        # on-device constants (no deps, run behind the DMA)
        eps_sb = consts.tile([32, 1], f32)
        nc.vector.memset(eps_sb, EPS)
        ebias_sb = consts.tile([C, 1], f32)
        nc.vector.memset(ebias_sb, EXP_BIAS)
        # big memsets go to gpsimd: the early DVE must be free for bn_stats
        wz = consts.tile([C, 512], f16)
        nc.gpsimd.memset(wz, 0.0)
        # sel16[p, b*128+j] = (p == b): selector rows for the r broadcast
        ones16 = consts.tile([16, 16 * C], f16)
        nc.gpsimd.memset(ones16, 1.0)
        sel16 = consts.tile([16, 16 * C], f16)
        nc.gpsimd.affine_select(
            out=sel16, in_=ones16, pattern=[[-1, 16], [0, C]],
            compare_op=OP.is_equal, fill=0.0, base=0, channel_multiplier=1)
        # identity for the PE transpose, built on device
        onesf = consts.tile([C, C], f32)
        nc.gpsimd.memset(onesf, 1.0)
        ident_sb = consts.tile([C, C], f32)
        nc.gpsimd.affine_select(
            out=ident_sb, in_=onesf, pattern=[[-1, C]],
            compare_op=OP.is_equal, fill=0.0, base=0, channel_multiplier=1)
        # ---- GroupNorm stats: per-channel mean/E[x2], combine 4ch/group via PE
        stats = work.tile([C, 8, 6], f32)
        for ch in range(8):
            nc.vector.bn_stats(out=stats[:, ch, :],
                               in_=x16[:, ch * 512:(ch + 1) * 512])
        mv = work.tile([C, 2], f32)
        nc.vector.bn_aggr(out=mv, in_=stats)
        # mv becomes (mean, E[x^2]) in place: col1 = mean^2 + var
        nc.vector.scalar_tensor_tensor(
            out=mv[:, 1:2], in0=mv[:, 0:1], scalar=mv[:, 0:1],
            in1=mv[:, 1:2], op0=OP.mult, op1=OP.add)
        # PE warmup: dummy matmuls gated on the first two DMA chunks run
        # entirely inside the DMA window, so the HAM clock-gate is released
        # before the stats chain and projections hit the PE.
        warm_ps = pproj.tile([C, 1024], f32, tag="pj")
        for w in range(4):
            nc.tensor.matmul(out=warm_ps[:, 0:512], lhsT=wz[:, 0:C],
                             rhs=x16[:, 0:512])
        for w in range(4):
            nc.tensor.matmul(out=warm_ps[:, 512:1024], lhsT=wz[:, 0:C],
                             rhs=x16[:, 1024:1536])
        gsum = pst.tile([32, 2], f32, tag="ps")
        nc.tensor.matmul(out=gsum, lhsT=gmap_sb, rhs=mv)  # (gmean, gex2)
        gmrs = work.tile([32, 2], f32)
        nc.vector.tensor_copy(out=gmrs[:, 0:1], in_=gsum[:, 0:1])
        # nvar = gmean^2 - gex2  (negated variance, fixed by Ln scale=-1)
        nvar = work.tile([32, 1], f32)
        nc.vector.scalar_tensor_tensor(
            out=nvar, in0=gmrs[:, 0:1], scalar=gmrs[:, 0:1], in1=gsum[:, 1:2],
            op0=OP.mult, op1=OP.subtract)
        gln = work.tile([32, 1], f32)
        nc.scalar.activation(out=gln, in_=nvar, func=AF.Ln, bias=eps_sb,
                             scale=-1.0)
        nc.scalar.activation(out=gmrs[:, 1:2], in_=gln, func=AF.Exp,
                             scale=-0.5)
        cstat = pst.tile([C, 2], f32, tag="ps")
        nc.tensor.matmul(out=cstat, lhsT=gmapt_sb, rhs=gmrs)  # (mean_c, rstd_c)
        # A = rstd_c * norm_w ; Bneg = mean_c * A - norm_b
        affA = work.tile([C, 1], f32)
        nc.vector.tensor_mul(out=affA, in0=cstat[:, 1:2], in1=nw_sb)
        bneg = work.tile([C, 1], f32)
        nc.vector.scalar_tensor_tensor(
            out=bneg, in0=cstat[:, 0:1], scalar=affA, in1=nb_sb,
            op0=OP.mult, op1=OP.subtract)
        bneg16 = work.tile([C, 1], f16)
        nc.vector.tensor_copy(out=bneg16, in_=bneg)

        # xh = A*x (keys + V input), hq = A*x - Bneg (queries)
        xh = big.tile([C, HW], f16)
        hq = big.tile([C, NQ], f16)
        # first chunk in 512s: the qk projection chain pipelines behind it
        for lo, hi in ((0, 512), (512, 1024), (1024, 2048)):
            nc.vector.tensor_scalar(
                out=hq[:, lo:hi], in0=x16[:, lo:hi],
                scalar1=affA, scalar2=bneg,
                op0=OP.mult, op1=OP.subtract)
        for ch in range(4):
            nc.vector.tensor_scalar_mul(
                out=xh[:, ch * 1024:(ch + 1) * 1024],
                in0=x16[:, ch * 1024:(ch + 1) * 1024], scalar1=affA)

        # output-bias chain (off the critical path): ob2 = Wo^T Wv^T Bneg
        pb = pst.tile([C, 1], f32, tag="ps")
        nc.tensor.matmul(out=pb, lhsT=wv_sb, rhs=bneg16)
        vb16 = work.tile([C, 1], f16)
        nc.vector.tensor_copy(out=vb16, in_=pb)
        pob = pst.tile([C, 1], f32, tag="ps")
        nc.tensor.matmul(out=pob, lhsT=wo_sb, rhs=vb16)
        obneg = work.tile([C, 1], f32)
        nc.vector.tensor_copy(out=obneg, in_=pob)

        qk = big.tile([C, NQ], f16)
        v_sb = big.tile([C, HW], f16)  # col block jb holds V[j, c] rows
        es = big.tile([C, NQ], f16)    # running exp-sum accumulator

        def qk_chunk(t, split=False):
            ps = pproj.tile([C, 1024], f32, tag="pj", name=f"qkps{t}")
            for k in range(2):
                nc.tensor.matmul(out=ps[:, k * 512:(k + 1) * 512],
                                 lhsT=wkq_sb,
                                 rhs=hq[:, t * 1024 + k * 512:
                                        t * 1024 + (k + 1) * 512])
            # evacuate on ScalarE: idle before the exp stream starts; the
            # first chunk in 512s so the first score matmul starts sooner
            if split:
                for k in range(2):
                    sl = slice(t * 1024 + k * 512, t * 1024 + (k + 1) * 512)
                    nc.scalar.copy(out=qk[:, sl],
                                   in_=ps[:, k * 512:(k + 1) * 512])
            else:
                nc.scalar.copy(out=qk[:, t * 1024:(t + 1) * 1024], in_=ps)

        def v_chunk(t, act=False):
            ps = pproj.tile([C, 1024], f32, tag="pj", name=f"vps{t}")
            for k in range(8):
                jb2 = t * 8 + k
                nc.tensor.matmul(out=ps[:, k * 128:(k + 1) * 128],
                                 lhsT=xh[:, jb2 * 128:(jb2 + 1) * 128],
                                 rhs=wv_sb)
            if act:
                nc.scalar.copy(out=v_sb[:, t * 1024:(t + 1) * 1024], in_=ps)
            else:
                nc.vector.tensor_copy(out=v_sb[:, t * 1024:(t + 1) * 1024],
                                      in_=ps)

        e_tiles = {}

        def emit_st_exp(jb):
            e_t = epool.tile([C, NQ], f16, tag="e", bufs=11, name=f"e{jb}")
            halves = (1, 0) if _dve_half(jb, 0) else (0, 1)
            for half in halves:
                st = pst.tile([C, HALF], f32, tag="ps", name=f"st{half}_{jb}")
                for k in range(2):
                    nc.tensor.matmul(
                        out=st[:, k * 512:(k + 1) * 512],
                        lhsT=xh[:, jb * 128:(jb + 1) * 128],
                        rhs=qk[:, half * HALF + k * 512:
                               half * HALF + (k + 1) * 512])
                dst = e_t[:, half * HALF:(half + 1) * HALF]
                if _dve_half(jb, half):
                    nc.vector.tensor_scalar(
                        out=dst.bitcast(i16), in0=st,
                        scalar1=SCH_B0, scalar2=0.0,
                        op0=OP.add, op1=OP.max)
                elif jb == 0:
                    # 512-wide pieces off the prologue's serial chain
                    for k in range(2):
                        sk = slice(k * 512, (k + 1) * 512)
                        nc.scalar.activation(out=dst[:, sk], in_=st[:, sk],
                                             func=AF.Exp, bias=ebias_sb,
                                             scale=1.0 / SCH_A)
                else:
                    nc.scalar.activation(out=dst, in_=st, func=AF.Exp,
                                         bias=ebias_sb, scale=1.0 / SCH_A)
            e_tiles[jb] = e_t

        # Pre-phase: qk + scores/exp for jb 0..7 interleaved with the V
        # projections.  (v chunk t covers key blocks 8t..8t+7.)
        qk_chunk(0, split=True)
        qk_chunk(1)
        emit_st_exp(0)
        v_chunk(0, act=True)
        emit_st_exp(1)
        emit_st_exp(2)
        v_chunk(1, act=True)
        emit_st_exp(3)
        emit_st_exp(4)
        v_chunk(2)
        emit_st_exp(5)
        emit_st_exp(6)
        v_chunk(3)
        emit_st_exp(7)
        pproj.release()
        pot = tc.alloc_tile_pool(name="pot", bufs=1, space="PSUM")

        def emit_es(jb):
            # per-half 1024-wide adds: finer DVE granularity keeps the
            # offloaded exp tiles' tensor_scalar from queueing behind a
            # long add, and the half-chains let scol start per half
            e_t = e_tiles[jb]
            for h in range(2):
                sl = slice(h * HALF, (h + 1) * HALF)
                if jb == 0:
                    nc.vector.tensor_copy(out=es[:, sl], in_=e_t[:, sl])
                else:
                    nc.vector.tensor_add(out=es[:, sl], in0=es[:, sl],
                                         in1=e_t[:, sl])

        def emit_pv(jb):
            for half in range(2):
                for k in range(2):
                    nc.tensor.matmul(
                        out=oT[:, half * HALF + k * 512:
                               half * HALF + (k + 1) * 512],
                        lhsT=v_sb[:, jb * 128:(jb + 1) * 128],
                        rhs=e_tiles[jb][:, half * HALF + k * 512:
                                        half * HALF + (k + 1) * 512],
                        start=(jb == 0), stop=(jb == JB - 1))

        # Steady loop, software-pipelined: scores run 8 key blocks ahead of
        # PV; the PV backlog drains with two groups on every third iteration.
        oT = pot.tile([C, NQ], f32, tag="ot")
        pv_next = 0
        for jb in range(JB):
            if jb + 8 < JB:
                emit_st_exp(jb + 8)
            n_pv = 2 if (jb % 3 == 2 and jb < 24) else 1
            for _ in range(n_pv):
                if pv_next < JB and pv_next <= jb + 7:
                    emit_pv(pv_next)
                    pv_next += 1
            emit_es(jb)
        while pv_next < JB:
            emit_pv(pv_next)
            pv_next += 1

        # ---- epilogue, per-half pipelined: denominator (ones-matmuls over
        # es plus the folded blocks 30/31 straight from their e tiles) ->
        # reciprocal -> PE transpose -> selector-matmul broadcast; the
        # normalization fuses into the oT evacuation (onrm = oT * r), then
        # output projection and fused residual-add, DMA per 512 columns.
        scols, r16s, onrms = [], [], []
        for half in range(2):
            scol = pst.tile([C, 8], f32, tag="ps", name=f"scol{half}")
            for m in range(8):
                ib = half * 8 + m
                nc.tensor.matmul(out=scol[:, m:m + 1],
                                 lhsT=es[:, ib * 128:(ib + 1) * 128],
                                 rhs=onesc_sb)
            scols.append(scol)
        for half in range(2):
            r_col = work.tile([C, 8], f32, name=f"rcol{half}")
            nc.vector.reciprocal(out=r_col, in_=scols[half])
            r16_ps = pst.tile([8, C], f32, tag="ps", name=f"r16ps{half}")
            nc.tensor.transpose(out=r16_ps, in_=r_col, identity=ident_sb)
            r16 = work.tile([8, C], f16, name=f"r16_{half}")
            nc.vector.tensor_copy(out=r16, in_=r16_ps)
            r16s.append(r16)
        for half in range(2):
            rbc = pst.tile([C, HALF], f32, tag="ps", name=f"rbc{half}")
            for m in range(8):
                nc.tensor.matmul(out=rbc[:, m * 128:(m + 1) * 128],
                                 lhsT=sel16[0:8, m * C:(m + 1) * C],
                                 rhs=r16s[half])
            # rc and the fused normalization in 512s: each output-projection
            # matmul starts as soon as its half-tile is ready
            rc_sb = work.tile([C, HALF], f16, name=f"rc{half}")
            onrm = big.tile([C, HALF], f16, name=f"onrm{half}")
            for k in range(2):
                sk = slice(k * 512, (k + 1) * 512)
                nc.scalar.copy(out=rc_sb[:, sk], in_=rbc[:, sk])
                nc.vector.tensor_mul(
                    out=onrm[:, sk],
                    in0=oT[:, half * HALF + k * 512:
                           half * HALF + (k + 1) * 512],
                    in1=rc_sb[:, sk])
            onrms.append(onrm)
        for half in range(2):
            op_ps = pst.tile([C, HALF], f32, tag="ps", name=f"op{half}")
            for k in range(2):
                nc.tensor.matmul(out=op_ps[:, k * 512:(k + 1) * 512],
                                 lhsT=wo_sb,
                                 rhs=onrms[half][:, k * 512:(k + 1) * 512])
            for k in range(2):
                i0 = half * HALF + k * 512
                y_sb = ypool.tile([C, 512], f16, name=f"y{half}_{k}")
                nc.vector.scalar_tensor_tensor(
                    out=y_sb, in0=op_ps[:, k * 512:(k + 1) * 512],
                    scalar=obneg,
                    in1=x16[:, i0:i0 + 512],
                    op0=OP.subtract, op1=OP.add)
                eng = nc.sync if k % 2 == 0 else nc.scalar
                eng.dma_start(out=y_d.ap()[:, i0:i0 + 512], in_=y_sb)

        for p in (pot, pst, ypool, epool, work, big, consts):
            p.release()

    nc.compile()
    return nc


def _get_nc():
    global _NC
    if _NC is None:
        _NC = _build_program()
    return _NC


def _make_packs(inputs):
    wq = np.asarray(inputs["Wq"], dtype=np.float64)
    wk = np.asarray(inputs["Wk"], dtype=np.float64)
    # lhsT for qk = G^T with G = (Wk Wq^T) * C^-0.5 * SCH_A
    wkq = (wq @ wk.T) * (C ** -0.5) * SCH_A
    wpack = np.zeros((C, _WPACK_W), np.float16)
    wpack[:, _WKQ:_WKQ + C] = wkq.astype(np.float16)
    wpack[:, _WV:_WV + C] = np.asarray(inputs["Wv"], np.float32).astype(np.float16)
    wpack[:, _WO:_WO + C] = np.asarray(inputs["Wo"], np.float32).astype(np.float16)
    wpack[:, _ONESC:_ONESC + 1] = 1.0
    gmap = np.zeros((C, 32), np.float32)
    for c in range(C):
        gmap[c, c // 4] = 0.25
    fpack = np.zeros((C, _FPACK_W), np.float32)
    fpack[:, _NW] = np.asarray(inputs["norm_w"], dtype=np.float32)
    fpack[:, _NB] = np.asarray(inputs["norm_b"], dtype=np.float32)
    fpack[:, _GMAP:_GMAP + 32] = gmap
    fpack[0:32, _GMAPT:_GMAPT + C] = np.sign(gmap.T)
    return wpack, fpack


def _make_in_maps(inputs):
    x = np.asarray(inputs["x"], dtype=np.float32).astype(np.float16)
    B = x.shape[0]
    xf = x.reshape(B, C, HW)
    wpack, fpack = _make_packs(inputs)
    in_maps = []
    for core in range(N_CORES):
        b, s = core // 2, core % 2
        xb = xf[b]
        if s == 1:
            xb = np.concatenate([xb[:, NQ:], xb[:, :NQ]], axis=1)
        in_maps.append({
            "x": np.ascontiguousarray(xb),
            "wpack": wpack, "fpack": fpack,
        })
    return in_maps


def _run_once(nc, in_maps):
    from concourse.bass_utils import run_bass_kernel_spmd

    res = run_bass_kernel_spmd(nc, in_maps, list(range(N_CORES)))
    return np.stack([np.asarray(res.results[core]["y"])
                     for core in range(N_CORES)])


def kernel(**inputs):
    nc = _get_nc()
    in_maps = _make_in_maps(inputs)
    # The kernel is deterministic, but a fresh NEFF's first execution has
    # been observed (rarely) to return corrupted data.  Re-execute until
    # two runs agree bit-for-bit (a corrupted run never reproduces).
    ys = _run_once(nc, in_maps)
    for _ in range(3):
        ys2 = _run_once(nc, in_maps)
        if np.array_equal(ys, ys2):
            break
        ys = ys2
    x = np.asarray(inputs["x"], dtype=np.float32)
    B, _, H, W = x.shape
    out = np.empty((B, C, HW), np.float32)
    for core in range(N_CORES):
        b, s = core // 2, core % 2
        out[b, :, s * NQ:(s + 1) * NQ] = ys[core].astype(np.float32)
    return out.reshape(B, C, H, W)


# revision 50
# speedup vs baseline: 1.0083x; 1.0083x over previous
"""AttnBlock (GroupNorm + single-head self-attention + residual) on 8 NeuronCores.

Sharding: data-parallel over B (4 batches) x sequence-parallel over query
rows (2 halves of H*W=4096) = 8 shards, one per core.  Each core loads its
batch's full x[b] as [C=128, HW=4096] fp16 (channels on partitions), with
the spatial columns rotated so the core's query half is cols [0:2048).

Key algebraic restructurings vs a direct lowering:
- GroupNorm folds to a per-channel affine h = A*x + B.  A is applied once
  to x (xh = A*x, one DVE op); the K-side B is dropped (softmax row
  invariance), the Q-side B rides in hq = A*x - Bneg, the V-side B folds
  into a constant output bias.
- The K projection is eliminated entirely: scores st[j,i] = k_j . q_i =
  xh_j^T (Wk Wq^T) h_i, so a single host-side product G = Wq Wk^T (scaled
  by C^-0.5 and the Schraudolph constant SCH_A) turns the whole Q/K path
  into one projection qk = G^T hq and score matmuls directly against xh
  blocks as stationary weights.
- Scores carry the x1477 Schraudolph pre-scale, so the DVE exp tiles are a
  SINGLE tensor_scalar (add bias, clamp at 0, int16 convert = fp16 bit
  pattern); ACT exp tiles undo the scale for free via the activation's
  fused scale/bias.

Main loop (32 key blocks x 2 query halves of 1024): scores transposed
into fp32 PSUM, exp to a per-block [128,2048] e tile (both halves), PV
accumulates oT[c,i] with V stationary.  Exp runs mostly on ScalarE; 13
halves run on VectorE via the one-op Schraudolph.  The e-sum runs as
per-half 1024-wide fp16 adds on VectorE (finer ops keep the offloaded
exp tiles' tensor_scalar from queueing behind a long add, which would
hold a score PSUM slot and stall ScalarE; 2048-wide adds and GpSimd
offload both measured slower).

Epilogue per half: denominator from es via ones-matmuls -> VectorE
reciprocal -> PE transpose -> selector-matmul broadcast; the
normalization fuses into the oT PSUM evacuation, then output projection
and a fused residual-add straight from PSUM, DMA per 512 columns.

kernel() re-executes until two runs agree bit-for-bit: a fresh NEFF's
first execution has (rarely) returned corrupted data, and a corrupted
run never reproduces.
"""

import numpy as np

C = 128
HW = 4096
NQ = 2048
HALF = 1024
JB = 32
EXP_BIAS = -8.0
EPS = 1e-5
N_CORES = 8

# Schraudolph fp16 exp: bits = round(st + SCH_B0), st = s*SCH_A pre-scaled
SCH_A = 1024.0 / float(np.log(2.0))
SCH_DELTA = -44.2
SCH_B0 = 15360.0 + SCH_DELTA + EXP_BIAS * SCH_A

# wpack (f16) column offsets
_WKQ, _WV, _WO, _ONESC = 0, 128, 256, 384
_WPACK_W = 385
# fpack (f32) column offsets
_NW, _NB, _GMAP, _GMAPT = 0, 1, 2, 34
_FPACK_W = 34 + 128

# which (jb, half) exp tiles run on VectorE (one-op Schraudolph); late
# blocks stay on ScalarE so the denominator chain is never DVE-gated
_DVE_SET = ({(jb, 0) for jb in (1, 6, 9, 11, 16, 19, 21, 26)}
            | {(jb, 1) for jb in (3, 8, 13, 18, 23)})

_NC = None


def _dve_half(jb, half):
    return (jb, half) in _DVE_SET


def _pin_activation_tables():
    """Restrict the table-load chooser to natural_log_exp_and_others so the
    kernel's ACT stream (ln/exp/copy/identity) needs a single table load."""
    from concourse.hw_specs import get_activation_tables
    tabs = get_activation_tables("gen3")
    for name in list(tabs.keys()):
        if name != "natural_log_exp_and_others":
            tabs[name] = set()


def _build_program():
    import concourse.bacc as bacc
    import concourse.tile as tile
    from concourse import mybir

    f32 = mybir.dt.float32
    f16 = mybir.dt.float16
    i16 = mybir.dt.int16
    AF = mybir.ActivationFunctionType
    OP = mybir.AluOpType

    nc = bacc.Bacc("TRN2", target_bir_lowering=False, debug=False,
                   num_devices=N_CORES)
    try:
        _pin_activation_tables()
    except Exception:
        pass

    x_d = nc.declare_dram_parameter("x", [C, HW], f16, isOutput=False)
    wpack_d = nc.declare_dram_parameter("wpack", [C, _WPACK_W], f16,
                                        isOutput=False)
    fpack_d = nc.declare_dram_parameter("fpack", [C, _FPACK_W], f32,
                                        isOutput=False)
    y_d = nc.declare_dram_parameter("y", [C, NQ], f16, isOutput=True)

    with tile.TileContext(nc) as tc:
        consts = tc.alloc_tile_pool(name="consts", bufs=1)
        big = tc.alloc_tile_pool(name="big", bufs=1)
        work = tc.alloc_tile_pool(name="work", bufs=2)
        epool = tc.alloc_tile_pool(name="epool", bufs=3)
        ypool = tc.alloc_tile_pool(name="ypool", bufs=4)
        pst = tc.alloc_tile_pool(name="pst", bufs=2, space="PSUM")
        pproj = tc.alloc_tile_pool(name="pproj", bufs=2, space="PSUM")

        # ---- input DMA: x in 4 chunks across both HWDGE rings, packs on
        # the gpsimd ring (fpack first: the stats chain needs it)
        x16 = big.tile([C, HW], f16)
        for ch in range(4):
            eng = nc.sync if ch % 2 == 0 else nc.scalar
            eng.dma_start(out=x16[:, ch * 1024:(ch + 1) * 1024],
                          in_=x_d.ap()[:, ch * 1024:(ch + 1) * 1024])
        fpack_sb = consts.tile([C, _FPACK_W], f32)
        nc.gpsimd.dma_start(out=fpack_sb, in_=fpack_d.ap())
        wpack_sb = consts.tile([C, _WPACK_W], f16)
        nc.gpsimd.dma_start(out=wpack_sb, in_=wpack_d.ap())
        wkq_sb = wpack_sb[:, _WKQ:_WKQ + C]    # lhsT for qk: (Wq Wk^T)*scale
        wv_sb = wpack_sb[:, _WV:_WV + C]
        wo_sb = wpack_sb[:, _WO:_WO + C]
        onesc_sb = wpack_sb[:, _ONESC:_ONESC + 1]
        nw_sb = fpack_sb[:, _NW:_NW + 1]
        nb_sb = fpack_sb[:, _NB:_NB + 1]
        gmap_sb = fpack_sb[:, _GMAP:_GMAP + 32]
        gmapt_sb = fpack_sb[0:32, _GMAPT:_GMAPT + C]

        # on-device constants (no deps, run behind the DMA)
        eps_sb = consts.tile([32, 1], f32)
        nc.vector.memset(eps_sb, EPS)
        ebias_sb = consts.tile([C, 1], f32)
        nc.vector.memset(ebias_sb, EXP_BIAS)
        # big memsets go to gpsimd: the early DVE must be free for bn_stats
        wz = consts.tile([C, 512], f16)
        nc.gpsimd.memset(wz, 0.0)
        # sel16[p, b*128+j] = (p == b): selector rows for the r broadcast
        ones16 = consts.tile([16, 16 * C], f16)
        nc.gpsimd.memset(ones16, 1.0)
        sel16 = consts.tile([16, 16 * C], f16)
        nc.gpsimd.affine_select(
            out=sel16, in_=ones16, pattern=[[-1, 16], [0, C]],
            compare_op=OP.is_equal, fill=0.0, base=0, channel_multiplier=1)
        # identity for the PE transpose, built on device
        onesf = consts.tile([C, C], f32)
        nc.gpsimd.memset(onesf, 1.0)
        ident_sb = consts.tile([C, C], f32)
        nc.gpsimd.affine_select(
            out=ident_sb, in_=onesf, pattern=[[-1, C]],
            compare_op=OP.is_equal, fill=0.0, base=0, channel_multiplier=1)
        # ---- GroupNorm stats: per-channel mean/E[x2], combine 4ch/group via PE
        stats = work.tile([C, 8, 6], f32)
        for ch in range(8):
            nc.vector.bn_stats(out=stats[:, ch, :],
                               in_=x16[:, ch * 512:(ch + 1) * 512])
        mv = work.tile([C, 2], f32)
        nc.vector.bn_aggr(out=mv, in_=stats)
        # mv becomes (mean, E[x^2]) in place: col1 = mean^2 + var
        nc.vector.scalar_tensor_tensor(
            out=mv[:, 1:2], in0=mv[:, 0:1], scalar=mv[:, 0:1],
            in1=mv[:, 1:2], op0=OP.mult, op1=OP.add)
        # PE warmup: dummy matmuls gated on the first two DMA chunks run
        # entirely inside the DMA window, so the HAM clock-gate is released
        # before the stats chain and projections hit the PE.
        warm_ps = pproj.tile([C, 1024], f32, tag="pj")
        for w in range(4):
            nc.tensor.matmul(out=warm_ps[:, 0:512], lhsT=wz[:, 0:C],
                             rhs=x16[:, 0:512])
        for w in range(4):
            nc.tensor.matmul(out=warm_ps[:, 512:1024], lhsT=wz[:, 0:C],
                             rhs=x16[:, 1024:1536])
        gsum = pst.tile([32, 2], f32, tag="ps")
        nc.tensor.matmul(out=gsum, lhsT=gmap_sb, rhs=mv)  # (gmean, gex2)
        gmrs = work.tile([32, 2], f32)
        nc.vector.tensor_copy(out=gmrs[:, 0:1], in_=gsum[:, 0:1])
        # nvar = gmean^2 - gex2  (negated variance, fixed by Ln scale=-1)
        nvar = work.tile([32, 1], f32)
        nc.vector.scalar_tensor_tensor(
            out=nvar, in0=gmrs[:, 0:1], scalar=gmrs[:, 0:1], in1=gsum[:, 1:2],
            op0=OP.mult, op1=OP.subtract)
        gln = work.tile([32, 1], f32)
        nc.scalar.activation(out=gln, in_=nvar, func=AF.Ln, bias=eps_sb,
                             scale=-1.0)
        nc.scalar.activation(out=gmrs[:, 1:2], in_=gln, func=AF.Exp,
                             scale=-0.5)
        cstat = pst.tile([C, 2], f32, tag="ps")
        nc.tensor.matmul(out=cstat, lhsT=gmapt_sb, rhs=gmrs)  # (mean_c, rstd_c)
        # A = rstd_c * norm_w ; Bneg = mean_c * A - norm_b
        affA = work.tile([C, 1], f32)
        nc.vector.tensor_mul(out=affA, in0=cstat[:, 1:2], in1=nw_sb)
        bneg = work.tile([C, 1], f32)
        nc.vector.scalar_tensor_tensor(
            out=bneg, in0=cstat[:, 0:1], scalar=affA, in1=nb_sb,
            op0=OP.mult, op1=OP.subtract)
        bneg16 = work.tile([C, 1], f16)
        nc.vector.tensor_copy(out=bneg16, in_=bneg)

        # xh = A*x (keys + V input), hq = A*x - Bneg (queries)
        xh = big.tile([C, HW], f16)
        hq = big.tile([C, NQ], f16)
        for ch in range(2):
            nc.vector.tensor_scalar(
                out=hq[:, ch * 1024:(ch + 1) * 1024],
                in0=x16[:, ch * 1024:(ch + 1) * 1024],
                scalar1=affA, scalar2=bneg,
                op0=OP.mult, op1=OP.subtract)
        for ch in range(4):
            nc.vector.tensor_scalar_mul(
                out=xh[:, ch * 1024:(ch + 1) * 1024],
                in0=x16[:, ch * 1024:(ch + 1) * 1024], scalar1=affA)

        # output-bias chain (off the critical path): ob2 = Wo^T Wv^T Bneg
        pb = pst.tile([C, 1], f32, tag="ps")
        nc.tensor.matmul(out=pb, lhsT=wv_sb, rhs=bneg16)
        vb16 = work.tile([C, 1], f16)
        nc.vector.tensor_copy(out=vb16, in_=pb)
        pob = pst.tile([C, 1], f32, tag="ps")
        nc.tensor.matmul(out=pob, lhsT=wo_sb, rhs=vb16)
        obneg = work.tile([C, 1], f32)
        nc.vector.tensor_copy(out=obneg, in_=pob)

        qk = big.tile([C, NQ], f16)
        v_sb = big.tile([C, HW], f16)  # col block jb holds V[j, c] rows
        es = big.tile([C, NQ], f16)    # running exp-sum accumulator

        def qk_chunk(t):
            ps = pproj.tile([C, 1024], f32, tag="pj", name=f"qkps{t}")
            for k in range(2):
                nc.tensor.matmul(out=ps[:, k * 512:(k + 1) * 512],
                                 lhsT=wkq_sb,
                                 rhs=hq[:, t * 1024 + k * 512:
                                        t * 1024 + (k + 1) * 512])
            # evacuate on ScalarE: idle before the exp stream starts
            nc.scalar.copy(out=qk[:, t * 1024:(t + 1) * 1024], in_=ps)

        def v_chunk(t, act=False):
            ps = pproj.tile([C, 1024], f32, tag="pj", name=f"vps{t}")
            for k in range(8):
                jb2 = t * 8 + k
                nc.tensor.matmul(out=ps[:, k * 128:(k + 1) * 128],
                                 lhsT=xh[:, jb2 * 128:(jb2 + 1) * 128],
                                 rhs=wv_sb)
            if act:
                nc.scalar.copy(out=v_sb[:, t * 1024:(t + 1) * 1024], in_=ps)
            else:
                nc.vector.tensor_copy(out=v_sb[:, t * 1024:(t + 1) * 1024],
                                      in_=ps)

        e_tiles = {}

        def emit_st_exp(jb):
            e_t = epool.tile([C, NQ], f16, tag="e", bufs=11, name=f"e{jb}")
            halves = (1, 0) if _dve_half(jb, 0) else (0, 1)
            for half in halves:
                st = pst.tile([C, HALF], f32, tag="ps", name=f"st{half}_{jb}")
                for k in range(2):
                    nc.tensor.matmul(
                        out=st[:, k * 512:(k + 1) * 512],
                        lhsT=xh[:, jb * 128:(jb + 1) * 128],
                        rhs=qk[:, half * HALF + k * 512:
                               half * HALF + (k + 1) * 512])
                dst = e_t[:, half * HALF:(half + 1) * HALF]
                if _dve_half(jb, half):
                    nc.vector.tensor_scalar(
                        out=dst.bitcast(i16), in0=st,
                        scalar1=SCH_B0, scalar2=0.0,
                        op0=OP.add, op1=OP.max)
                else:
                    nc.scalar.activation(out=dst, in_=st, func=AF.Exp,
                                         bias=ebias_sb, scale=1.0 / SCH_A)
            e_tiles[jb] = e_t

        # Pre-phase: qk + scores/exp for jb 0..7 interleaved with the V
        # projections.  (v chunk t covers key blocks 8t..8t+7.)
        qk_chunk(0)
        qk_chunk(1)
        emit_st_exp(0)
        v_chunk(0, act=True)
        emit_st_exp(1)
        emit_st_exp(2)
        v_chunk(1, act=True)
        emit_st_exp(3)
        emit_st_exp(4)
        v_chunk(2)
        emit_st_exp(5)
        emit_st_exp(6)
        v_chunk(3)
        emit_st_exp(7)
        pproj.release()
        pot = tc.alloc_tile_pool(name="pot", bufs=1, space="PSUM")

        def emit_es(jb):
            # per-half 1024-wide adds: finer DVE granularity keeps the
            # offloaded exp tiles' tensor_scalar from queueing behind a
            # long add, and the half-chains let scol start per half
            e_t = e_tiles[jb]
            for h in range(2):
                sl = slice(h * HALF, (h + 1) * HALF)
                if jb == 0:
                    nc.vector.tensor_copy(out=es[:, sl], in_=e_t[:, sl])
                else:
                    nc.vector.tensor_add(out=es[:, sl], in0=es[:, sl],
                                         in1=e_t[:, sl])

        def emit_pv(jb):
            for half in range(2):
                for k in range(2):
                    nc.tensor.matmul(
                        out=oT[:, half * HALF + k * 512:
                               half * HALF + (k + 1) * 512],
                        lhsT=v_sb[:, jb * 128:(jb + 1) * 128],
                        rhs=e_tiles[jb][:, half * HALF + k * 512:
                                        half * HALF + (k + 1) * 512],
                        start=(jb == 0), stop=(jb == JB - 1))

        # Steady loop, software-pipelined: scores run 8 key blocks ahead of
        # PV; the PV backlog drains with two groups on every third iteration.
        oT = pot.tile([C, NQ], f32, tag="ot")
        pv_next = 0
        for jb in range(JB):
            if jb + 8 < JB:
                emit_st_exp(jb + 8)
            n_pv = 2 if (jb % 3 == 2 and jb < 24) else 1
            for _ in range(n_pv):
                if pv_next < JB and pv_next <= jb + 7:
                    emit_pv(pv_next)
                    pv_next += 1
            emit_es(jb)
        while pv_next < JB:
            emit_pv(pv_next)
            pv_next += 1

        # ---- epilogue, per-half pipelined: denominator (ones-matmuls over
        # es plus the folded blocks 30/31 straight from their e tiles) ->
        # reciprocal -> PE transpose -> selector-matmul broadcast; the
        # normalization fuses into the oT evacuation (onrm = oT * r), then
        # output projection and fused residual-add, DMA per 512 columns.
        scols, r16s, onrms = [], [], []
        for half in range(2):
            scol = pst.tile([C, 8], f32, tag="ps", name=f"scol{half}")
            for m in range(8):
                ib = half * 8 + m
                nc.tensor.matmul(out=scol[:, m:m + 1],
                                 lhsT=es[:, ib * 128:(ib + 1) * 128],
                                 rhs=onesc_sb)
            scols.append(scol)
        for half in range(2):
            r_col = work.tile([C, 8], f32, name=f"rcol{half}")
            nc.vector.reciprocal(out=r_col, in_=scols[half])
            r16_ps = pst.tile([8, C], f32, tag="ps", name=f"r16ps{half}")
            nc.tensor.transpose(out=r16_ps, in_=r_col, identity=ident_sb)
            r16 = work.tile([8, C], f16, name=f"r16_{half}")
            nc.vector.tensor_copy(out=r16, in_=r16_ps)
            r16s.append(r16)
        for half in range(2):
            rbc = pst.tile([C, HALF], f32, tag="ps", name=f"rbc{half}")
            for m in range(8):
                nc.tensor.matmul(out=rbc[:, m * 128:(m + 1) * 128],
                                 lhsT=sel16[0:8, m * C:(m + 1) * C],
                                 rhs=r16s[half])
            rc_sb = work.tile([C, HALF], f16, name=f"rc{half}")
            nc.scalar.copy(out=rc_sb, in_=rbc)
            onrm = big.tile([C, HALF], f16, name=f"onrm{half}")
            nc.vector.tensor_mul(
                out=onrm, in0=oT[:, half * HALF:(half + 1) * HALF],
                in1=rc_sb)
            onrms.append(onrm)
        for half in range(2):
            op_ps = pst.tile([C, HALF], f32, tag="ps", name=f"op{half}")
            for k in range(2):
                nc.tensor.matmul(out=op_ps[:, k * 512:(k + 1) * 512],
                                 lhsT=wo_sb,
                                 rhs=onrms[half][:, k * 512:(k + 1) * 512])
            for k in range(2):
                i0 = half * HALF + k * 512
                y_sb = ypool.tile([C, 512], f16, name=f"y{half}_{k}")
                nc.vector.scalar_tensor_tensor(
                    out=y_sb, in0=op_ps[:, k * 512:(k + 1) * 512],
                    scalar=obneg,
                    in1=x16[:, i0:i0 + 512],
                    op0=OP.subtract, op1=OP.add)
                eng = nc.sync if k % 2 == 0 else nc.scalar
                eng.dma_start(out=y_d.ap()[:, i0:i0 + 512], in_=y_sb)

        for p in (pot, pst, ypool, epool, work, big, consts):
            p.release()

    nc.compile()
    return nc


def _get_nc():
    global _NC
    if _NC is None:
        _NC = _build_program()
    return _NC


def _make_packs(inputs):
    wq = np.asarray(inputs["Wq"], dtype=np.float64)
    wk = np.asarray(inputs["Wk"], dtype=np.float64)
    # lhsT for qk = G^T with G = (Wk Wq^T) * C^-0.5 * SCH_A
    wkq = (wq @ wk.T) * (C ** -0.5) * SCH_A
    wpack = np.zeros((C, _WPACK_W), np.float16)
    wpack[:, _WKQ:_WKQ + C] = wkq.astype(np.float16)
    wpack[:, _WV:_WV + C] = np.asarray(inputs["Wv"], np.float32).astype(np.float16)
    wpack[:, _WO:_WO + C] = np.asarray(inputs["Wo"], np.float32).astype(np.float16)
    wpack[:, _ONESC:_ONESC + 1] = 1.0
    gmap = np.zeros((C, 32), np.float32)
    for c in range(C):
        gmap[c, c // 4] = 0.25
    fpack = np.zeros((C, _FPACK_W), np.float32)
    fpack[:, _NW] = np.asarray(inputs["norm_w"], dtype=np.float32)
    fpack[:, _NB] = np.asarray(inputs["norm_b"], dtype=np.float32)
    fpack[:, _GMAP:_GMAP + 32] = gmap
    fpack[0:32, _GMAPT:_GMAPT + C] = np.sign(gmap.T)
    return wpack, fpack


def _make_in_maps(inputs):
    x = np.asarray(inputs["x"], dtype=np.float32).astype(np.float16)
    B = x.shape[0]
    xf = x.reshape(B, C, HW)
    wpack, fpack = _make_packs(inputs)
    in_maps = []
    for core in range(N_CORES):
        b, s = core // 2, core % 2
        xb = xf[b]
        if s == 1:
            xb = np.concatenate([xb[:, NQ:], xb[:, :NQ]], axis=1)
        in_maps.append({
            "x": np.ascontiguousarray(xb),
            "wpack": wpack, "fpack": fpack,
        })
    return in_maps


def _run_once(nc, in_maps):
    from concourse.bass_utils import run_bass_kernel_spmd

    res = run_bass_kernel_spmd(nc, in_maps, list(range(N_CORES)))
    return np.stack([np.asarray(res.results[core]["y"])
                     for core in range(N_CORES)])


def kernel(**inputs):
    nc = _get_nc()
    in_maps = _make_in_maps(inputs)
    # The kernel is deterministic, but a fresh NEFF's first execution has
    # been observed (rarely) to return corrupted data.  Re-execute until
    # two runs agree bit-for-bit (a corrupted run never reproduces).
    ys = _run_once(nc, in_maps)
    for _ in range(3):
        ys2 = _run_once(nc, in_maps)
        if np.array_equal(ys, ys2):
            break
        ys = ys2
    x = np.asarray(inputs["x"], dtype=np.float32)
    B, _, H, W = x.shape
    out = np.empty((B, C, HW), np.float32)
    for core in range(N_CORES):
        b, s = core // 2, core % 2
        out[b, :, s * NQ:(s + 1) * NQ] = ys[core].astype(np.float32)
    return out.reshape(B, C, H, W)


# revision 51
# speedup vs baseline: 1.0110x; 1.0027x over previous
"""AttnBlock (GroupNorm + single-head self-attention + residual) on 8 NeuronCores.

Sharding: data-parallel over B (4 batches) x sequence-parallel over query
rows (2 halves of H*W=4096) = 8 shards, one per core.  Each core loads its
batch's full x[b] as [C=128, HW=4096] fp16 (channels on partitions), with
the spatial columns rotated so the core's query half is cols [0:2048).

Key algebraic restructurings vs a direct lowering:
- GroupNorm folds to a per-channel affine h = A*x + B.  A is applied once
  to x (xh = A*x, one DVE op); the K-side B is dropped (softmax row
  invariance), the Q-side B rides in hq = A*x - Bneg, the V-side B folds
  into a constant output bias.
- The K projection is eliminated entirely: scores st[j,i] = k_j . q_i =
  xh_j^T (Wk Wq^T) h_i, so a single host-side product G = Wq Wk^T (scaled
  by C^-0.5 and the Schraudolph constant SCH_A) turns the whole Q/K path
  into one projection qk = G^T hq and score matmuls directly against xh
  blocks as stationary weights.
- Scores carry the x1477 Schraudolph pre-scale, so the DVE exp tiles are a
  SINGLE tensor_scalar (add bias, clamp at 0, int16 convert = fp16 bit
  pattern); ACT exp tiles undo the scale for free via the activation's
  fused scale/bias.

Main loop (32 key blocks x 2 query halves of 1024): scores transposed
into fp32 PSUM, exp to a per-block [128,2048] e tile (both halves), PV
accumulates oT[c,i] with V stationary.  Exp runs mostly on ScalarE; 13
halves run on VectorE via the one-op Schraudolph.  The e-sum runs as
per-half 1024-wide fp16 adds on VectorE (finer ops keep the offloaded
exp tiles' tensor_scalar from queueing behind a long add, which would
hold a score PSUM slot and stall ScalarE; 2048-wide adds and GpSimd
offload both measured slower).

Epilogue per half: denominator from es via ones-matmuls -> VectorE
reciprocal -> PE transpose -> selector-matmul broadcast; the
normalization fuses into the oT PSUM evacuation, then output projection
and a fused residual-add straight from PSUM, DMA per 512 columns.

kernel() re-executes until two runs agree bit-for-bit: a fresh NEFF's
first execution has (rarely) returned corrupted data, and a corrupted
run never reproduces.
"""

import numpy as np

C = 128
HW = 4096
NQ = 2048
HALF = 1024
JB = 32
EXP_BIAS = -8.0
EPS = 1e-5
N_CORES = 8

# Schraudolph fp16 exp: bits = round(st + SCH_B0), st = s*SCH_A pre-scaled
SCH_A = 1024.0 / float(np.log(2.0))
SCH_DELTA = -44.2
SCH_B0 = 15360.0 + SCH_DELTA + EXP_BIAS * SCH_A

# wpack (f16) column offsets
_WKQ, _WV, _WO, _ONESC = 0, 128, 256, 384
_WPACK_W = 385
# fpack (f32) column offsets
_NW, _NB, _GMAP, _GMAPT = 0, 1, 2, 34
_FPACK_W = 34 + 128

# which (jb, half) exp tiles run on VectorE (one-op Schraudolph); sized
# so ScalarE stays the clear pacer and VectorE keeps slack (es adds +
# 10 tiles < ACT stream), late blocks stay on ScalarE so the denominator
# chain is never DVE-gated
_DVE_SET = ({(jb, 0) for jb in (1, 6, 11, 16, 21, 26)}
            | {(jb, 1) for jb in (3, 8, 18, 23)})

_NC = None


def _dve_half(jb, half):
    return (jb, half) in _DVE_SET


def _pin_activation_tables():
    """Restrict the table-load chooser to natural_log_exp_and_others so the
    kernel's ACT stream (ln/exp/copy/identity) needs a single table load."""
    from concourse.hw_specs import get_activation_tables
    tabs = get_activation_tables("gen3")
    for name in list(tabs.keys()):
        if name != "natural_log_exp_and_others":
            tabs[name] = set()


def _build_program():
    import concourse.bacc as bacc
    import concourse.tile as tile
    from concourse import mybir

    f32 = mybir.dt.float32
    f16 = mybir.dt.float16
    i16 = mybir.dt.int16
    AF = mybir.ActivationFunctionType
    OP = mybir.AluOpType

    nc = bacc.Bacc("TRN2", target_bir_lowering=False, debug=False,
                   num_devices=N_CORES)
    try:
        _pin_activation_tables()
    except Exception:
        pass

    x_d = nc.declare_dram_parameter("x", [C, HW], f16, isOutput=False)
    wpack_d = nc.declare_dram_parameter("wpack", [C, _WPACK_W], f16,
                                        isOutput=False)
    fpack_d = nc.declare_dram_parameter("fpack", [C, _FPACK_W], f32,
                                        isOutput=False)
    y_d = nc.declare_dram_parameter("y", [C, NQ], f16, isOutput=True)

    with tile.TileContext(nc) as tc:
        consts = tc.alloc_tile_pool(name="consts", bufs=1)
        big = tc.alloc_tile_pool(name="big", bufs=1)
        work = tc.alloc_tile_pool(name="work", bufs=2)
        epool = tc.alloc_tile_pool(name="epool", bufs=3)
        ypool = tc.alloc_tile_pool(name="ypool", bufs=4)
        pst = tc.alloc_tile_pool(name="pst", bufs=2, space="PSUM")
        pproj = tc.alloc_tile_pool(name="pproj", bufs=2, space="PSUM")

        # ---- input DMA: x in 4 chunks across both HWDGE rings, packs on
        # the gpsimd ring (fpack first: the stats chain needs it)
        x16 = big.tile([C, HW], f16)
        for ch in range(4):
            eng = nc.sync if ch % 2 == 0 else nc.scalar
            eng.dma_start(out=x16[:, ch * 1024:(ch + 1) * 1024],
                          in_=x_d.ap()[:, ch * 1024:(ch + 1) * 1024])
        fpack_sb = consts.tile([C, _FPACK_W], f32)
        nc.gpsimd.dma_start(out=fpack_sb, in_=fpack_d.ap())
        wpack_sb = consts.tile([C, _WPACK_W], f16)
        nc.gpsimd.dma_start(out=wpack_sb, in_=wpack_d.ap())
        wkq_sb = wpack_sb[:, _WKQ:_WKQ + C]    # lhsT for qk: (Wq Wk^T)*scale
        wv_sb = wpack_sb[:, _WV:_WV + C]
        wo_sb = wpack_sb[:, _WO:_WO + C]
        onesc_sb = wpack_sb[:, _ONESC:_ONESC + 1]
        nw_sb = fpack_sb[:, _NW:_NW + 1]
        nb_sb = fpack_sb[:, _NB:_NB + 1]
        gmap_sb = fpack_sb[:, _GMAP:_GMAP + 32]
        gmapt_sb = fpack_sb[0:32, _GMAPT:_GMAPT + C]

        # on-device constants (no deps, run behind the DMA)
        eps_sb = consts.tile([32, 1], f32)
        nc.vector.memset(eps_sb, EPS)
        ebias_sb = consts.tile([C, 1], f32)
        nc.vector.memset(ebias_sb, EXP_BIAS)
        # big memsets go to gpsimd: the early DVE must be free for bn_stats
        wz = consts.tile([C, 512], f16)
        nc.gpsimd.memset(wz, 0.0)
        # sel16[p, b*128+j] = (p == b): selector rows for the r broadcast
        ones16 = consts.tile([16, 16 * C], f16)
        nc.gpsimd.memset(ones16, 1.0)
        sel16 = consts.tile([16, 16 * C], f16)
        nc.gpsimd.affine_select(
            out=sel16, in_=ones16, pattern=[[-1, 16], [0, C]],
            compare_op=OP.is_equal, fill=0.0, base=0, channel_multiplier=1)
        # identity for the PE transpose, built on device
        onesf = consts.tile([C, C], f32)
        nc.gpsimd.memset(onesf, 1.0)
        ident_sb = consts.tile([C, C], f32)
        nc.gpsimd.affine_select(
            out=ident_sb, in_=onesf, pattern=[[-1, C]],
            compare_op=OP.is_equal, fill=0.0, base=0, channel_multiplier=1)
        # ---- GroupNorm stats: per-channel mean/E[x2], combine 4ch/group via PE
        stats = work.tile([C, 8, 6], f32)
        for ch in range(8):
            nc.vector.bn_stats(out=stats[:, ch, :],
                               in_=x16[:, ch * 512:(ch + 1) * 512])
        mv = work.tile([C, 2], f32)
        nc.vector.bn_aggr(out=mv, in_=stats)
        # mv becomes (mean, E[x^2]) in place: col1 = mean^2 + var
        nc.vector.scalar_tensor_tensor(
            out=mv[:, 1:2], in0=mv[:, 0:1], scalar=mv[:, 0:1],
            in1=mv[:, 1:2], op0=OP.mult, op1=OP.add)
        # PE warmup: dummy matmuls gated on the first two DMA chunks run
        # entirely inside the DMA window, so the HAM clock-gate is released
        # before the stats chain and projections hit the PE.
        warm_ps = pproj.tile([C, 1024], f32, tag="pj")
        for w in range(4):
            nc.tensor.matmul(out=warm_ps[:, 0:512], lhsT=wz[:, 0:C],
                             rhs=x16[:, 0:512])
        for w in range(4):
            nc.tensor.matmul(out=warm_ps[:, 512:1024], lhsT=wz[:, 0:C],
                             rhs=x16[:, 1024:1536])
        gsum = pst.tile([32, 2], f32, tag="ps")
        nc.tensor.matmul(out=gsum, lhsT=gmap_sb, rhs=mv)  # (gmean, gex2)
        gmrs = work.tile([32, 2], f32)
        nc.vector.tensor_copy(out=gmrs[:, 0:1], in_=gsum[:, 0:1])
        # nvar = gmean^2 - gex2  (negated variance, fixed by Ln scale=-1)
        nvar = work.tile([32, 1], f32)
        nc.vector.scalar_tensor_tensor(
            out=nvar, in0=gmrs[:, 0:1], scalar=gmrs[:, 0:1], in1=gsum[:, 1:2],
            op0=OP.mult, op1=OP.subtract)
        gln = work.tile([32, 1], f32)
        nc.scalar.activation(out=gln, in_=nvar, func=AF.Ln, bias=eps_sb,
                             scale=-1.0)
        nc.scalar.activation(out=gmrs[:, 1:2], in_=gln, func=AF.Exp,
                             scale=-0.5)
        cstat = pst.tile([C, 2], f32, tag="ps")
        nc.tensor.matmul(out=cstat, lhsT=gmapt_sb, rhs=gmrs)  # (mean_c, rstd_c)
        # A = rstd_c * norm_w ; Bneg = mean_c * A - norm_b
        affA = work.tile([C, 1], f32)
        nc.vector.tensor_mul(out=affA, in0=cstat[:, 1:2], in1=nw_sb)
        bneg = work.tile([C, 1], f32)
        nc.vector.scalar_tensor_tensor(
            out=bneg, in0=cstat[:, 0:1], scalar=affA, in1=nb_sb,
            op0=OP.mult, op1=OP.subtract)
        bneg16 = work.tile([C, 1], f16)
        nc.vector.tensor_copy(out=bneg16, in_=bneg)

        # xh = A*x (keys + V input), hq = A*x - Bneg (queries)
        xh = big.tile([C, HW], f16)
        hq = big.tile([C, NQ], f16)
        for ch in range(2):
            nc.vector.tensor_scalar(
                out=hq[:, ch * 1024:(ch + 1) * 1024],
                in0=x16[:, ch * 1024:(ch + 1) * 1024],
                scalar1=affA, scalar2=bneg,
                op0=OP.mult, op1=OP.subtract)
        for ch in range(4):
            nc.vector.tensor_scalar_mul(
                out=xh[:, ch * 1024:(ch + 1) * 1024],
                in0=x16[:, ch * 1024:(ch + 1) * 1024], scalar1=affA)

        # output-bias chain (off the critical path): ob2 = Wo^T Wv^T Bneg
        pb = pst.tile([C, 1], f32, tag="ps")
        nc.tensor.matmul(out=pb, lhsT=wv_sb, rhs=bneg16)
        vb16 = work.tile([C, 1], f16)
        nc.vector.tensor_copy(out=vb16, in_=pb)
        pob = pst.tile([C, 1], f32, tag="ps")
        nc.tensor.matmul(out=pob, lhsT=wo_sb, rhs=vb16)
        obneg = work.tile([C, 1], f32)
        nc.vector.tensor_copy(out=obneg, in_=pob)

        qk = big.tile([C, NQ], f16)
        v_sb = big.tile([C, HW], f16)  # col block jb holds V[j, c] rows
        es = big.tile([C, NQ], f16)    # running exp-sum accumulator

        def qk_chunk(t):
            ps = pproj.tile([C, 1024], f32, tag="pj", name=f"qkps{t}")
            for k in range(2):
                nc.tensor.matmul(out=ps[:, k * 512:(k + 1) * 512],
                                 lhsT=wkq_sb,
                                 rhs=hq[:, t * 1024 + k * 512:
                                        t * 1024 + (k + 1) * 512])
            # evacuate on ScalarE: idle before the exp stream starts
            nc.scalar.copy(out=qk[:, t * 1024:(t + 1) * 1024], in_=ps)

        def v_chunk(t, act=False):
            ps = pproj.tile([C, 1024], f32, tag="pj", name=f"vps{t}")
            for k in range(8):
                jb2 = t * 8 + k
                nc.tensor.matmul(out=ps[:, k * 128:(k + 1) * 128],
                                 lhsT=xh[:, jb2 * 128:(jb2 + 1) * 128],
                                 rhs=wv_sb)
            if act:
                nc.scalar.copy(out=v_sb[:, t * 1024:(t + 1) * 1024], in_=ps)
            else:
                nc.vector.tensor_copy(out=v_sb[:, t * 1024:(t + 1) * 1024],
                                      in_=ps)

        e_tiles = {}

        def emit_st_exp(jb):
            e_t = epool.tile([C, NQ], f16, tag="e", bufs=11, name=f"e{jb}")
            halves = (1, 0) if _dve_half(jb, 0) else (0, 1)
            for half in halves:
                st = pst.tile([C, HALF], f32, tag="ps", name=f"st{half}_{jb}")
                for k in range(2):
                    nc.tensor.matmul(
                        out=st[:, k * 512:(k + 1) * 512],
                        lhsT=xh[:, jb * 128:(jb + 1) * 128],
                        rhs=qk[:, half * HALF + k * 512:
                               half * HALF + (k + 1) * 512])
                dst = e_t[:, half * HALF:(half + 1) * HALF]
                if _dve_half(jb, half):
                    nc.vector.tensor_scalar(
                        out=dst.bitcast(i16), in0=st,
                        scalar1=SCH_B0, scalar2=0.0,
                        op0=OP.add, op1=OP.max)
                else:
                    nc.scalar.activation(out=dst, in_=st, func=AF.Exp,
                                         bias=ebias_sb, scale=1.0 / SCH_A)
            e_tiles[jb] = e_t

        # Pre-phase: qk + scores/exp for jb 0..7 interleaved with the V
        # projections.  (v chunk t covers key blocks 8t..8t+7.)
        qk_chunk(0)
        qk_chunk(1)
        emit_st_exp(0)
        v_chunk(0, act=True)
        emit_st_exp(1)
        emit_st_exp(2)
        v_chunk(1, act=True)
        emit_st_exp(3)
        emit_st_exp(4)
        v_chunk(2)
        emit_st_exp(5)
        emit_st_exp(6)
        v_chunk(3)
        emit_st_exp(7)
        pproj.release()
        pot = tc.alloc_tile_pool(name="pot", bufs=1, space="PSUM")

        def emit_es(jb):
            # per-half 1024-wide adds: finer DVE granularity keeps the
            # offloaded exp tiles' tensor_scalar from queueing behind a
            # long add, and the half-chains let scol start per half
            e_t = e_tiles[jb]
            for h in range(2):
                sl = slice(h * HALF, (h + 1) * HALF)
                if jb == 0:
                    nc.vector.tensor_copy(out=es[:, sl], in_=e_t[:, sl])
                else:
                    nc.vector.tensor_add(out=es[:, sl], in0=es[:, sl],
                                         in1=e_t[:, sl])

        def emit_pv(jb):
            for half in range(2):
                for k in range(2):
                    nc.tensor.matmul(
                        out=oT[:, half * HALF + k * 512:
                               half * HALF + (k + 1) * 512],
                        lhsT=v_sb[:, jb * 128:(jb + 1) * 128],
                        rhs=e_tiles[jb][:, half * HALF + k * 512:
                                        half * HALF + (k + 1) * 512],
                        start=(jb == 0), stop=(jb == JB - 1))

        # Steady loop, software-pipelined: scores run 8 key blocks ahead of
        # PV; the PV backlog drains with two groups on every third iteration.
        oT = pot.tile([C, NQ], f32, tag="ot")
        pv_next = 0
        for jb in range(JB):
            if jb + 8 < JB:
                emit_st_exp(jb + 8)
            n_pv = 2 if (jb % 3 == 2 and jb < 24) else 1
            for _ in range(n_pv):
                if pv_next < JB and pv_next <= jb + 7:
                    emit_pv(pv_next)
                    pv_next += 1
            emit_es(jb)
        while pv_next < JB:
            emit_pv(pv_next)
            pv_next += 1

        # ---- epilogue, per-half pipelined: denominator (ones-matmuls over
        # es plus the folded blocks 30/31 straight from their e tiles) ->
        # reciprocal -> PE transpose -> selector-matmul broadcast; the
        # normalization fuses into the oT evacuation (onrm = oT * r), then
        # output projection and fused residual-add, DMA per 512 columns.
        scols, r16s, onrms = [], [], []
        for half in range(2):
            scol = pst.tile([C, 8], f32, tag="ps", name=f"scol{half}")
            for m in range(8):
                ib = half * 8 + m
                nc.tensor.matmul(out=scol[:, m:m + 1],
                                 lhsT=es[:, ib * 128:(ib + 1) * 128],
                                 rhs=onesc_sb)
            scols.append(scol)
        for half in range(2):
            r_col = work.tile([C, 8], f32, name=f"rcol{half}")
            nc.vector.reciprocal(out=r_col, in_=scols[half])
            r16_ps = pst.tile([8, C], f32, tag="ps", name=f"r16ps{half}")
            nc.tensor.transpose(out=r16_ps, in_=r_col, identity=ident_sb)
            r16 = work.tile([8, C], f16, name=f"r16_{half}")
            nc.vector.tensor_copy(out=r16, in_=r16_ps)
            r16s.append(r16)
        for half in range(2):
            rbc = pst.tile([C, HALF], f32, tag="ps", name=f"rbc{half}")
            for m in range(8):
                nc.tensor.matmul(out=rbc[:, m * 128:(m + 1) * 128],
                                 lhsT=sel16[0:8, m * C:(m + 1) * C],
                                 rhs=r16s[half])
            rc_sb = work.tile([C, HALF], f16, name=f"rc{half}")
            nc.scalar.copy(out=rc_sb, in_=rbc)
            onrm = big.tile([C, HALF], f16, name=f"onrm{half}")
            nc.vector.tensor_mul(
                out=onrm, in0=oT[:, half * HALF:(half + 1) * HALF],
                in1=rc_sb)
            onrms.append(onrm)
        for half in range(2):
            op_ps = pst.tile([C, HALF], f32, tag="ps", name=f"op{half}")
            for k in range(2):
                nc.tensor.matmul(out=op_ps[:, k * 512:(k + 1) * 512],
                                 lhsT=wo_sb,
                                 rhs=onrms[half][:, k * 512:(k + 1) * 512])
            for k in range(2):
                i0 = half * HALF + k * 512
                y_sb = ypool.tile([C, 512], f16, name=f"y{half}_{k}")
                nc.vector.scalar_tensor_tensor(
                    out=y_sb, in0=op_ps[:, k * 512:(k + 1) * 512],
                    scalar=obneg,
                    in1=x16[:, i0:i0 + 512],
                    op0=OP.subtract, op1=OP.add)
                eng = nc.sync if k % 2 == 0 else nc.scalar
                eng.dma_start(out=y_d.ap()[:, i0:i0 + 512], in_=y_sb)

        for p in (pot, pst, ypool, epool, work, big, consts):
            p.release()

    nc.compile()
    return nc


def _get_nc():
    global _NC
    if _NC is None:
        _NC = _build_program()
    return _NC


def _make_packs(inputs):
    wq = np.asarray(inputs["Wq"], dtype=np.float64)
    wk = np.asarray(inputs["Wk"], dtype=np.float64)
    # lhsT for qk = G^T with G = (Wk Wq^T) * C^-0.5 * SCH_A
    wkq = (wq @ wk.T) * (C ** -0.5) * SCH_A
    wpack = np.zeros((C, _WPACK_W), np.float16)
    wpack[:, _WKQ:_WKQ + C] = wkq.astype(np.float16)
    wpack[:, _WV:_WV + C] = np.asarray(inputs["Wv"], np.float32).astype(np.float16)
    wpack[:, _WO:_WO + C] = np.asarray(inputs["Wo"], np.float32).astype(np.float16)
    wpack[:, _ONESC:_ONESC + 1] = 1.0
    gmap = np.zeros((C, 32), np.float32)
    for c in range(C):
        gmap[c, c // 4] = 0.25
    fpack = np.zeros((C, _FPACK_W), np.float32)
    fpack[:, _NW] = np.asarray(inputs["norm_w"], dtype=np.float32)
    fpack[:, _NB] = np.asarray(inputs["norm_b"], dtype=np.float32)
    fpack[:, _GMAP:_GMAP + 32] = gmap
    fpack[0:32, _GMAPT:_GMAPT + C] = np.sign(gmap.T)
    return wpack, fpack


def _make_in_maps(inputs):
    x = np.asarray(inputs["x"], dtype=np.float32).astype(np.float16)
    B = x.shape[0]
    xf = x.reshape(B, C, HW)
    wpack, fpack = _make_packs(inputs)
    in_maps = []
    for core in range(N_CORES):
        b, s = core // 2, core % 2
        xb = xf[b]
        if s == 1:
            xb = np.concatenate([xb[:, NQ:], xb[:, :NQ]], axis=1)
        in_maps.append({
            "x": np.ascontiguousarray(xb),
            "wpack": wpack, "fpack": fpack,
        })
    return in_maps


def _run_once(nc, in_maps):
    from concourse.bass_utils import run_bass_kernel_spmd

    res = run_bass_kernel_spmd(nc, in_maps, list(range(N_CORES)))
    return np.stack([np.asarray(res.results[core]["y"])
                     for core in range(N_CORES)])


def kernel(**inputs):
    nc = _get_nc()
    in_maps = _make_in_maps(inputs)
    # The kernel is deterministic, but a fresh NEFF's first execution has
    # been observed (rarely) to return corrupted data.  Re-execute until
    # two runs agree bit-for-bit (a corrupted run never reproduces).
    ys = _run_once(nc, in_maps)
    for _ in range(3):
        ys2 = _run_once(nc, in_maps)
        if np.array_equal(ys, ys2):
            break
        ys = ys2
    x = np.asarray(inputs["x"], dtype=np.float32)
    B, _, H, W = x.shape
    out = np.empty((B, C, HW), np.float32)
    for core in range(N_CORES):
        b, s = core // 2, core % 2
        out[b, :, s * NQ:(s + 1) * NQ] = ys[core].astype(np.float32)
    return out.reshape(B, C, H, W)


# revision 52
# speedup vs baseline: 1.0156x; 1.0046x over previous
"""AttnBlock (GroupNorm + single-head self-attention + residual) on 8 NeuronCores.

Sharding: data-parallel over B (4 batches) x sequence-parallel over query
rows (2 halves of H*W=4096) = 8 shards, one per core.  Each core loads its
batch's full x[b] as [C=128, HW=4096] fp16 (channels on partitions), with
the spatial columns rotated so the core's query half is cols [0:2048).

Key algebraic restructurings vs a direct lowering:
- GroupNorm folds to a per-channel affine h = A*x + B.  A is applied once
  to x (xh = A*x, one DVE op); the K-side B is dropped (softmax row
  invariance), the Q-side B rides in hq = A*x - Bneg, the V-side B folds
  into a constant output bias.
- The K projection is eliminated entirely: scores st[j,i] = k_j . q_i =
  xh_j^T (Wk Wq^T) h_i, so a single host-side product G = Wq Wk^T (scaled
  by C^-0.5 and the Schraudolph constant SCH_A) turns the whole Q/K path
  into one projection qk = G^T hq and score matmuls directly against xh
  blocks as stationary weights.
- Scores carry the x1477 Schraudolph pre-scale, so the DVE exp tiles are a
  SINGLE tensor_scalar (add bias, clamp at 0, int16 convert = fp16 bit
  pattern); ACT exp tiles undo the scale for free via the activation's
  fused scale/bias.

Main loop (32 key blocks x 2 query halves of 1024): scores transposed
into fp32 PSUM, exp to a per-block [128,2048] e tile (both halves), PV
accumulates oT[c,i] with V stationary.  Exp runs mostly on ScalarE; 13
halves run on VectorE via the one-op Schraudolph.  The e-sum runs as
per-half 1024-wide fp16 adds on VectorE (finer ops keep the offloaded
exp tiles' tensor_scalar from queueing behind a long add, which would
hold a score PSUM slot and stall ScalarE; 2048-wide adds and GpSimd
offload both measured slower).

Epilogue per half: denominator from es via ones-matmuls -> VectorE
reciprocal -> PE transpose -> selector-matmul broadcast; the
normalization fuses into the oT PSUM evacuation, then output projection
and a fused residual-add straight from PSUM, DMA per 512 columns.

kernel() re-executes until two runs agree bit-for-bit: a fresh NEFF's
first execution has (rarely) returned corrupted data, and a corrupted
run never reproduces.
"""

import numpy as np

C = 128
HW = 4096
NQ = 2048
HALF = 1024
JB = 32
EXP_BIAS = -8.0
EPS = 1e-5
N_CORES = 8

# Schraudolph fp16 exp: bits = round(st + SCH_B0), st = s*SCH_A pre-scaled
SCH_A = 1024.0 / float(np.log(2.0))
SCH_DELTA = -44.2
SCH_B0 = 15360.0 + SCH_DELTA + EXP_BIAS * SCH_A

# wpack (f16) column offsets
_WKQ, _WV, _WO, _ONESC = 0, 128, 256, 384
_WPACK_W = 385
# fpack (f32) column offsets
_NW, _NB, _GMAP, _GMAPT = 0, 1, 2, 34
_FPACK_W = 34 + 128

# which (jb, half) exp tiles run on VectorE (one-op Schraudolph); late
# blocks stay on ScalarE so the denominator chain is never DVE-gated
_DVE_SET = ({(jb, 0) for jb in (1, 6, 9, 11, 16, 19, 21, 26)}
            | {(jb, 1) for jb in (3, 8, 13, 18, 23)})

_NC = None


def _dve_half(jb, half):
    return (jb, half) in _DVE_SET


def _pin_activation_tables():
    """Restrict the table-load chooser to natural_log_exp_and_others so the
    kernel's ACT stream (ln/exp/copy/identity) needs a single table load."""
    from concourse.hw_specs import get_activation_tables
    tabs = get_activation_tables("gen3")
    for name in list(tabs.keys()):
        if name != "natural_log_exp_and_others":
            tabs[name] = set()


def _build_program():
    import concourse.bacc as bacc
    import concourse.tile as tile
    from concourse import mybir

    f32 = mybir.dt.float32
    f16 = mybir.dt.float16
    i16 = mybir.dt.int16
    AF = mybir.ActivationFunctionType
    OP = mybir.AluOpType

    nc = bacc.Bacc("TRN2", target_bir_lowering=False, debug=False,
                   num_devices=N_CORES)
    try:
        _pin_activation_tables()
    except Exception:
        pass

    x_d = nc.declare_dram_parameter("x", [C, HW], f16, isOutput=False)
    wpack_d = nc.declare_dram_parameter("wpack", [C, _WPACK_W], f16,
                                        isOutput=False)
    fpack_d = nc.declare_dram_parameter("fpack", [C, _FPACK_W], f32,
                                        isOutput=False)
    y_d = nc.declare_dram_parameter("y", [C, NQ], f16, isOutput=True)

    with tile.TileContext(nc) as tc:
        consts = tc.alloc_tile_pool(name="consts", bufs=1)
        big = tc.alloc_tile_pool(name="big", bufs=1)
        work = tc.alloc_tile_pool(name="work", bufs=2)
        epool = tc.alloc_tile_pool(name="epool", bufs=3)
        ypool = tc.alloc_tile_pool(name="ypool", bufs=4)
        pst = tc.alloc_tile_pool(name="pst", bufs=2, space="PSUM")
        pproj = tc.alloc_tile_pool(name="pproj", bufs=2, space="PSUM")

        # ---- input DMA: x in 4 chunks across both HWDGE rings, packs on
        # the gpsimd ring (fpack first: the stats chain needs it)
        x16 = big.tile([C, HW], f16)
        for ch in range(4):
            eng = nc.sync if ch % 2 == 0 else nc.scalar
            eng.dma_start(out=x16[:, ch * 1024:(ch + 1) * 1024],
                          in_=x_d.ap()[:, ch * 1024:(ch + 1) * 1024])
        fpack_sb = consts.tile([C, _FPACK_W], f32)
        nc.gpsimd.dma_start(out=fpack_sb, in_=fpack_d.ap())
        wpack_sb = consts.tile([C, _WPACK_W], f16)
        nc.gpsimd.dma_start(out=wpack_sb, in_=wpack_d.ap())
        wkq_sb = wpack_sb[:, _WKQ:_WKQ + C]    # lhsT for qk: (Wq Wk^T)*scale
        wv_sb = wpack_sb[:, _WV:_WV + C]
        wo_sb = wpack_sb[:, _WO:_WO + C]
        onesc_sb = wpack_sb[:, _ONESC:_ONESC + 1]
        nw_sb = fpack_sb[:, _NW:_NW + 1]
        nb_sb = fpack_sb[:, _NB:_NB + 1]
        gmap_sb = fpack_sb[:, _GMAP:_GMAP + 32]
        gmapt_sb = fpack_sb[0:32, _GMAPT:_GMAPT + C]

        # on-device constants (no deps, run behind the DMA)
        eps_sb = consts.tile([32, 1], f32)
        nc.vector.memset(eps_sb, EPS)
        ebias_sb = consts.tile([C, 1], f32)
        nc.vector.memset(ebias_sb, EXP_BIAS)
        # big memsets go to gpsimd: the early DVE must be free for bn_stats
        wz = consts.tile([C, 512], f16)
        nc.gpsimd.memset(wz, 0.0)
        # sel16[p, b*128+j] = (p == b): selector rows for the r broadcast
        ones16 = consts.tile([16, 16 * C], f16)
        nc.gpsimd.memset(ones16, 1.0)
        sel16 = consts.tile([16, 16 * C], f16)
        nc.gpsimd.affine_select(
            out=sel16, in_=ones16, pattern=[[-1, 16], [0, C]],
            compare_op=OP.is_equal, fill=0.0, base=0, channel_multiplier=1)
        # identity for the PE transpose, built on device
        onesf = consts.tile([C, C], f32)
        nc.gpsimd.memset(onesf, 1.0)
        ident_sb = consts.tile([C, C], f32)
        nc.gpsimd.affine_select(
            out=ident_sb, in_=onesf, pattern=[[-1, C]],
            compare_op=OP.is_equal, fill=0.0, base=0, channel_multiplier=1)
        # ---- GroupNorm stats: per-channel mean/E[x2], combine 4ch/group via PE
        stats = work.tile([C, 8, 6], f32)
        for ch in range(8):
            nc.vector.bn_stats(out=stats[:, ch, :],
                               in_=x16[:, ch * 512:(ch + 1) * 512])
        mv = work.tile([C, 2], f32)
        nc.vector.bn_aggr(out=mv, in_=stats)
        # mv becomes (mean, E[x^2]) in place: col1 = mean^2 + var
        nc.vector.scalar_tensor_tensor(
            out=mv[:, 1:2], in0=mv[:, 0:1], scalar=mv[:, 0:1],
            in1=mv[:, 1:2], op0=OP.mult, op1=OP.add)
        # PE warmup: dummy matmuls gated on the first two DMA chunks run
        # entirely inside the DMA window, so the HAM clock-gate is released
        # before the stats chain and projections hit the PE.
        warm_ps = pproj.tile([C, 1024], f32, tag="pj")
        for w in range(4):
            nc.tensor.matmul(out=warm_ps[:, 0:512], lhsT=wz[:, 0:C],
                             rhs=x16[:, 0:512])
        for w in range(4):
            nc.tensor.matmul(out=warm_ps[:, 512:1024], lhsT=wz[:, 0:C],
                             rhs=x16[:, 1024:1536])
        gsum = pst.tile([32, 2], f32, tag="ps")
        nc.tensor.matmul(out=gsum, lhsT=gmap_sb, rhs=mv)  # (gmean, gex2)
        gmrs = work.tile([32, 2], f32)
        nc.vector.tensor_copy(out=gmrs[:, 0:1], in_=gsum[:, 0:1])
        # nvar = gmean^2 - gex2  (negated variance, fixed by Ln scale=-1)
        nvar = work.tile([32, 1], f32)
        nc.vector.scalar_tensor_tensor(
            out=nvar, in0=gmrs[:, 0:1], scalar=gmrs[:, 0:1], in1=gsum[:, 1:2],
            op0=OP.mult, op1=OP.subtract)
        gln = work.tile([32, 1], f32)
        nc.scalar.activation(out=gln, in_=nvar, func=AF.Ln, bias=eps_sb,
                             scale=-1.0)
        nc.scalar.activation(out=gmrs[:, 1:2], in_=gln, func=AF.Exp,
                             scale=-0.5)
        cstat = pst.tile([C, 2], f32, tag="ps")
        nc.tensor.matmul(out=cstat, lhsT=gmapt_sb, rhs=gmrs)  # (mean_c, rstd_c)
        # A = rstd_c * norm_w ; Bneg = mean_c * A - norm_b
        affA = work.tile([C, 1], f32)
        nc.vector.tensor_mul(out=affA, in0=cstat[:, 1:2], in1=nw_sb)
        bneg = work.tile([C, 1], f32)
        nc.vector.scalar_tensor_tensor(
            out=bneg, in0=cstat[:, 0:1], scalar=affA, in1=nb_sb,
            op0=OP.mult, op1=OP.subtract)
        bneg16 = work.tile([C, 1], f16)
        nc.vector.tensor_copy(out=bneg16, in_=bneg)

        # xh = A*x (keys + V input), hq = A*x - Bneg (queries)
        xh = big.tile([C, HW], f16)
        hq = big.tile([C, NQ], f16)
        for ch in range(2):
            nc.vector.tensor_scalar(
                out=hq[:, ch * 1024:(ch + 1) * 1024],
                in0=x16[:, ch * 1024:(ch + 1) * 1024],
                scalar1=affA, scalar2=bneg,
                op0=OP.mult, op1=OP.subtract)
        for ch in range(4):
            nc.vector.tensor_scalar_mul(
                out=xh[:, ch * 1024:(ch + 1) * 1024],
                in0=x16[:, ch * 1024:(ch + 1) * 1024], scalar1=affA)

        # output-bias chain (off the critical path): ob2 = Wo^T Wv^T Bneg
        pb = pst.tile([C, 1], f32, tag="ps")
        nc.tensor.matmul(out=pb, lhsT=wv_sb, rhs=bneg16)
        vb16 = work.tile([C, 1], f16)
        nc.vector.tensor_copy(out=vb16, in_=pb)
        pob = pst.tile([C, 1], f32, tag="ps")
        nc.tensor.matmul(out=pob, lhsT=wo_sb, rhs=vb16)
        obneg = work.tile([C, 1], f32)
        nc.vector.tensor_copy(out=obneg, in_=pob)

        qk = big.tile([C, NQ], f16)
        v_sb = big.tile([C, HW], f16)  # col block jb holds V[j, c] rows
        es = big.tile([C, NQ], f16)    # running exp-sum accumulator

        def qk_chunk(t):
            ps = pproj.tile([C, 1024], f32, tag="pj", name=f"qkps{t}")
            for k in range(2):
                nc.tensor.matmul(out=ps[:, k * 512:(k + 1) * 512],
                                 lhsT=wkq_sb,
                                 rhs=hq[:, t * 1024 + k * 512:
                                        t * 1024 + (k + 1) * 512])
            # evacuate on ScalarE: idle before the exp stream starts
            nc.scalar.copy(out=qk[:, t * 1024:(t + 1) * 1024], in_=ps)

        def v_chunk(t, act=False):
            ps = pproj.tile([C, 1024], f32, tag="pj", name=f"vps{t}")
            for k in range(8):
                jb2 = t * 8 + k
                nc.tensor.matmul(out=ps[:, k * 128:(k + 1) * 128],
                                 lhsT=xh[:, jb2 * 128:(jb2 + 1) * 128],
                                 rhs=wv_sb)
            if act:
                nc.scalar.copy(out=v_sb[:, t * 1024:(t + 1) * 1024], in_=ps)
            else:
                nc.vector.tensor_copy(out=v_sb[:, t * 1024:(t + 1) * 1024],
                                      in_=ps)

        e_tiles = {}

        def emit_st_exp(jb):
            e_t = epool.tile([C, NQ], f16, tag="e", bufs=11, name=f"e{jb}")
            halves = (1, 0) if _dve_half(jb, 0) else (0, 1)
            for half in halves:
                st = pst.tile([C, HALF], f32, tag="ps", name=f"st{half}_{jb}")
                for k in range(2):
                    nc.tensor.matmul(
                        out=st[:, k * 512:(k + 1) * 512],
                        lhsT=xh[:, jb * 128:(jb + 1) * 128],
                        rhs=qk[:, half * HALF + k * 512:
                               half * HALF + (k + 1) * 512])
                dst = e_t[:, half * HALF:(half + 1) * HALF]
                if _dve_half(jb, half):
                    nc.vector.tensor_scalar(
                        out=dst.bitcast(i16), in0=st,
                        scalar1=SCH_B0, scalar2=0.0,
                        op0=OP.add, op1=OP.max)
                else:
                    nc.scalar.activation(out=dst, in_=st, func=AF.Exp,
                                         bias=ebias_sb, scale=1.0 / SCH_A)
            e_tiles[jb] = e_t

        # Pre-phase: qk + scores/exp for jb 0..7 interleaved with the V
        # projections.  (v chunk t covers key blocks 8t..8t+7.)
        qk_chunk(0)
        qk_chunk(1)
        emit_st_exp(0)
        v_chunk(0, act=True)
        emit_st_exp(1)
        emit_st_exp(2)
        v_chunk(1, act=True)
        emit_st_exp(3)
        emit_st_exp(4)
        v_chunk(2)
        emit_st_exp(5)
        emit_st_exp(6)
        v_chunk(3)
        emit_st_exp(7)
        pproj.release()
        pot = tc.alloc_tile_pool(name="pot", bufs=1, space="PSUM")

        def emit_es(jb):
            # per-half 1024-wide adds: finer DVE granularity keeps the
            # offloaded exp tiles' tensor_scalar from queueing behind a
            # long add, and the half-chains let scol start per half
            e_t = e_tiles[jb]
            for h in range(2):
                sl = slice(h * HALF, (h + 1) * HALF)
                if jb == 0:
                    nc.vector.tensor_copy(out=es[:, sl], in_=e_t[:, sl])
                else:
                    nc.vector.tensor_add(out=es[:, sl], in0=es[:, sl],
                                         in1=e_t[:, sl])

        def emit_pv(jb):
            for half in range(2):
                for k in range(2):
                    nc.tensor.matmul(
                        out=oT[:, half * HALF + k * 512:
                               half * HALF + (k + 1) * 512],
                        lhsT=v_sb[:, jb * 128:(jb + 1) * 128],
                        rhs=e_tiles[jb][:, half * HALF + k * 512:
                                        half * HALF + (k + 1) * 512],
                        start=(jb == 0), stop=(jb == JB - 1))

        # Steady loop, software-pipelined: scores run 8 key blocks ahead of
        # PV; the PV backlog drains with two groups on every third iteration.
        oT = pot.tile([C, NQ], f32, tag="ot")
        pv_next = 0
        for jb in range(JB):
            if jb + 8 < JB:
                emit_st_exp(jb + 8)
            n_pv = 2 if (jb % 3 == 2 and jb < 24) else 1
            for _ in range(n_pv):
                if pv_next < JB and pv_next <= jb + 7:
                    emit_pv(pv_next)
                    pv_next += 1
            emit_es(jb)
        while pv_next < JB:
            emit_pv(pv_next)
            pv_next += 1

        # ---- epilogue, per-half pipelined: denominator (ones-matmuls over
        # es plus the folded blocks 30/31 straight from their e tiles) ->
        # reciprocal -> PE transpose -> selector-matmul broadcast; the
        # normalization fuses into the oT evacuation (onrm = oT * r), then
        # output projection and fused residual-add, DMA per 512 columns.
        scols, r16s, onrms = [], [], []
        for half in range(2):
            scol = pst.tile([C, 8], f32, tag="ps", name=f"scol{half}")
            for m in range(8):
                ib = half * 8 + m
                nc.tensor.matmul(out=scol[:, m:m + 1],
                                 lhsT=es[:, ib * 128:(ib + 1) * 128],
                                 rhs=onesc_sb)
            scols.append(scol)
        for half in range(2):
            r_col = work.tile([C, 8], f32, name=f"rcol{half}")
            nc.vector.reciprocal(out=r_col, in_=scols[half])
            r16_ps = pst.tile([8, C], f32, tag="ps", name=f"r16ps{half}")
            nc.tensor.transpose(out=r16_ps, in_=r_col, identity=ident_sb)
            r16 = work.tile([8, C], f16, name=f"r16_{half}")
            nc.vector.tensor_copy(out=r16, in_=r16_ps)
            r16s.append(r16)
        for half in range(2):
            rbc = pst.tile([C, HALF], f32, tag="ps", name=f"rbc{half}")
            for m in range(8):
                nc.tensor.matmul(out=rbc[:, m * 128:(m + 1) * 128],
                                 lhsT=sel16[0:8, m * C:(m + 1) * C],
                                 rhs=r16s[half])
            rc_sb = work.tile([C, HALF], f16, name=f"rc{half}")
            nc.scalar.copy(out=rc_sb, in_=rbc)
            onrm = big.tile([C, HALF], f16, name=f"onrm{half}")
            nc.vector.tensor_mul(
                out=onrm, in0=oT[:, half * HALF:(half + 1) * HALF],
                in1=rc_sb)
            onrms.append(onrm)
        for half in range(2):
            op_ps = pst.tile([C, HALF], f32, tag="ps", name=f"op{half}")
            for k in range(2):
                nc.tensor.matmul(out=op_ps[:, k * 512:(k + 1) * 512],
                                 lhsT=wo_sb,
                                 rhs=onrms[half][:, k * 512:(k + 1) * 512])
            for k in range(2):
                i0 = half * HALF + k * 512
                y_sb = ypool.tile([C, 512], f16, name=f"y{half}_{k}")
                nc.vector.scalar_tensor_tensor(
                    out=y_sb, in0=op_ps[:, k * 512:(k + 1) * 512],
                    scalar=obneg,
                    in1=x16[:, i0:i0 + 512],
                    op0=OP.subtract, op1=OP.add)
                eng = nc.sync if k % 2 == 0 else nc.scalar
                eng.dma_start(out=y_d.ap()[:, i0:i0 + 512], in_=y_sb)

        for p in (pot, pst, ypool, epool, work, big, consts):
            p.release()

    nc.compile()
    return nc


def _get_nc():
    global _NC
    if _NC is None:
        _NC = _build_program()
    return _NC


def _make_packs(inputs):
    wq = np.asarray(inputs["Wq"], dtype=np.float64)
    wk = np.asarray(inputs["Wk"], dtype=np.float64)
    # lhsT for qk = G^T with G = (Wk Wq^T) * C^-0.5 * SCH_A
    wkq = (wq @ wk.T) * (C ** -0.5) * SCH_A
    wpack = np.zeros((C, _WPACK_W), np.float16)
    wpack[:, _WKQ:_WKQ + C] = wkq.astype(np.float16)
    wpack[:, _WV:_WV + C] = np.asarray(inputs["Wv"], np.float32).astype(np.float16)
    wpack[:, _WO:_WO + C] = np.asarray(inputs["Wo"], np.float32).astype(np.float16)
    wpack[:, _ONESC:_ONESC + 1] = 1.0
    gmap = np.zeros((C, 32), np.float32)
    for c in range(C):
        gmap[c, c // 4] = 0.25
    fpack = np.zeros((C, _FPACK_W), np.float32)
    fpack[:, _NW] = np.asarray(inputs["norm_w"], dtype=np.float32)
    fpack[:, _NB] = np.asarray(inputs["norm_b"], dtype=np.float32)
    fpack[:, _GMAP:_GMAP + 32] = gmap
    fpack[0:32, _GMAPT:_GMAPT + C] = np.sign(gmap.T)
    return wpack, fpack


def _make_in_maps(inputs):
    x = np.asarray(inputs["x"], dtype=np.float32).astype(np.float16)
    B = x.shape[0]
    xf = x.reshape(B, C, HW)
    wpack, fpack = _make_packs(inputs)
    in_maps = []
    for core in range(N_CORES):
        b, s = core // 2, core % 2
        xb = xf[b]
        if s == 1:
            xb = np.concatenate([xb[:, NQ:], xb[:, :NQ]], axis=1)
        in_maps.append({
            "x": np.ascontiguousarray(xb),
            "wpack": wpack, "fpack": fpack,
        })
    return in_maps


def _run_once(nc, in_maps):
    from concourse.bass_utils import run_bass_kernel_spmd

    res = run_bass_kernel_spmd(nc, in_maps, list(range(N_CORES)))
    return np.stack([np.asarray(res.results[core]["y"])
                     for core in range(N_CORES)])


def kernel(**inputs):
    nc = _get_nc()
    in_maps = _make_in_maps(inputs)
    # The kernel is deterministic, but a fresh NEFF's first execution has
    # been observed (rarely) to return corrupted data.  Re-execute until
    # two runs agree bit-for-bit (a corrupted run never reproduces).
    ys = _run_once(nc, in_maps)
    for _ in range(3):
        ys2 = _run_once(nc, in_maps)
        if np.array_equal(ys, ys2):
            break
        ys = ys2
    x = np.asarray(inputs["x"], dtype=np.float32)
    B, _, H, W = x.shape
    out = np.empty((B, C, HW), np.float32)
    for core in range(N_CORES):
        b, s = core // 2, core % 2
        out[b, :, s * NQ:(s + 1) * NQ] = ys[core].astype(np.float32)
    return out.reshape(B, C, H, W)
